# revision 3
# baseline (speedup 1.0000x reference)
"""Trainium2 Bass kernel for nn_AttitudeController (B=2097152 drones).

Contract: kernel(**inputs) takes the FULL unsharded inputs (numpy) and
returns the FULL [B, 4] float32 output.  Internally the batch is sharded
across 8 NeuronCores; each core runs an identical NEFF on its shard.

Math (derived from the reference):
    R_des^T R = R(q_err),  q_err = q_y(th/2)* x q_x(ph/2)* x q_z(ps/2)* x q
    angle_error = [2ab, 2ac, 0]          (a,b,c,d = q_err components)
    M[:,2]      = [2(bd+ac), 2(cd-ab), 1-2(b^2+c^2)]
    rate_error  = ang_vel - yaw_rate * M[:,2]
    out[r] = sum_k Wf[r,k] * f_k - 1
Wf has +-uniform-magnitude columns for the quad-X mixer, so the final
stage folds into 4 group values G0..G3 and a sign butterfly.  Products
stay UNdoubled on-device; the factors of 2 are folded into constants.

v3 structure (this file): software-pipelined emission.  Each tile is
split into front (DMA + ACT sines + GpSimd extractions), body (the DVE
quaternion chain), and back (GpSimd output butterfly + DMA out).  Fronts
run two tiles ahead so no engine ever waits on a same-tile round trip:
  emit: F0 F1 | B0 K0 F2 | B1 K1 F3 | B2 K2 | B3 K3
Engine budget per 512-col tile (measured rates):
  ACT  ~8.3us: 2 sin-triples, psw, psw2b, G3
  Pool ~9.8us: q4/eav01/psw2a extractions, g2a, 4 output writes
  DVE ~16.9us: 3 quat stages, 6 products, M/Sg, s-terms, GB, UV
  DMA ~16.6us: rs in, ct in, out
"""

import hashlib
import math

import numpy as np

B_TOTAL = 2097152
N_CORES = 8
SHARD = B_TOTAL // N_CORES          # 262144 rows per core
P = 128                             # SBUF partitions
COLS = SHARD // P                   # 2048 columns per partition

# --- tunables -------------------------------------------------------------
COMPUTE_DT = "float16"              # intermediate dtype on-chip
TILE_WIDTHS = [512, 512, 512, 512]  # column tiling of the 2048 cols
IO_BUFS = 2
TMP_BUFS = 2
LOOKAHEAD = 2                       # how many tiles the front runs ahead
PARTIAL_OK = False
MAX_WAITS = 1                       # walrus (this build) allows 1 wait/inst

_PIO2 = float(np.float32(math.pi / 2.0))
_SQRT2 = float(np.float32(math.sqrt(2.0)))   # used by folded_numpy only

_CACHE = {}


# --------------------------------------------------------------------------
# BIR post-processing: this walrus build rejects >1 sync-wait per
# instruction; split offenders into preceding Drain instructions.
# --------------------------------------------------------------------------
_bir_patch_installed = False


def _split_waits_in_bir(bir_bytes):
    import orjson

    d = orjson.loads(bir_bytes)
    changed = False
    mods = d.get("modules", [d]) if "functions" not in d else [d]
    for mod in mods:
        for fn in mod.get("functions", []):
            for blk in fn.get("blocks", []):
                out = []
                for ins in blk.get("instructions", []):
                    si = ins.get("sync_info") or {}
                    waits = si.get("on_wait") or []
                    if len(waits) > MAX_WAITS:
                        changed = True
                        chunks = [
                            waits[i : i + MAX_WAITS]
                            for i in range(0, len(waits), MAX_WAITS)
                        ]
                        for k, ch in enumerate(chunks[:-1]):
                            pre = {
                                "name": f"{ins['name']}-wsplit{k}",
                                "opcode": "Drain",
                                "engine": ins.get("engine", "SP"),
                                "ins": [],
                                "outs": [],
                                "is_reset_sema": False,
                                "sync_info": {"on_update": [], "on_wait": ch},
                            }
                            if "debug" in ins:
                                pre["debug"] = ins["debug"]
                            out.append(pre)
                        si["on_wait"] = chunks[-1]
                        ins["sync_info"] = si
                    out.append(ins)
                blk["instructions"] = out
    if changed:
        return orjson.dumps(d)
    return bir_bytes


def _install_bir_patch():
    global _bir_patch_installed
    if _bir_patch_installed:
        return
    from concourse import bass_utils

    orig = bass_utils.compile_bir_kernel

    def patched(bir_json, tmpdir, neff_name="file.neff", **kw):
        bj = bir_json if isinstance(bir_json, (bytes, bytearray)) else bir_json.encode()
        return orig(_split_waits_in_bir(bytes(bj)), tmpdir, neff_name=neff_name, **kw)

    bass_utils.compile_bir_kernel = patched
    # bass2jax imported the symbol directly
    from concourse import bass2jax

    bass2jax.compile_bir_kernel = patched
    _bir_patch_installed = True


# --------------------------------------------------------------------------
# Parameter folding
# --------------------------------------------------------------------------
def _fold_params(mass, g, mixer, max_thrusts, gain_attitude, gain_angular_rate):
    mixer = np.asarray(mixer, np.float64)
    mt = np.asarray(max_thrusts, np.float64)
    ga = np.asarray(gain_attitude, np.float64)
    gar = np.asarray(gain_angular_rate, np.float64)
    m2 = 2.0 * mixer / mt[:, None]  # [4 rotors, 4]
    Wf = np.zeros((4, 6))
    Wf[:, 0] = -m2[:, 0] * ga[0]     # coeff of 2ab
    Wf[:, 1] = -m2[:, 1] * ga[1]     # coeff of 2ac
    Wf[:, 2] = -m2[:, 0] * gar[0]    # coeff of rate_err0
    Wf[:, 3] = -m2[:, 1] * gar[1]    # coeff of rate_err1
    Wf[:, 4] = -m2[:, 2] * gar[2]    # coeff of rate_err2
    Wf[:, 5] = m2[:, 3] * float(mass) * float(g)

    def col_mag(k):
        m = np.abs(Wf[:, k])
        if not np.allclose(m, m[0], rtol=1e-5):
            raise RuntimeError(f"mixer column {k} magnitudes not uniform: {m}")
        return float(m[0])

    wa, wa1, wr, wr1, wr2, wt = (col_mag(k) for k in range(6))
    if not (np.isclose(wa, wa1, rtol=1e-5) and np.isclose(wr, wr1, rtol=1e-5)):
        raise RuntimeError("roll/pitch gain magnitudes differ; v3 needs wa==wa1")
    sA = np.sign(Wf[:, 0]).astype(int)
    sB = np.sign(Wf[:, 1]).astype(int)
    sC = np.sign(Wf[:, 4]).astype(int)
    if not (np.sign(Wf[:, 2]) == sA).all():
        raise RuntimeError("columns 0/2 sign mismatch")
    if not (np.sign(Wf[:, 3]) == sB).all():
        raise RuntimeError("columns 1/3 sign mismatch")
    if not (np.sign(Wf[:, 5]) > 0).all():
        raise RuntimeError("thrust column must be positive")
    return dict(
        wa=wa, wa1=wa1, wr=wr, wr1=wr1, wr2=wr2, wt=wt,
        sA=sA.tolist(), sB=sB.tolist(), sC=sC.tolist(), Wf=Wf,
    )


def folded_numpy(root_state, control_target, fp):
    """Numpy model of exactly what the device computes (fp32). Used by
    test.py to validate the algebra separately from the hardware."""
    q = root_state[:, 3:7].astype(np.float32)
    av = root_state[:, 10:13].astype(np.float32)
    ph = control_target[:, 0]
    th = control_target[:, 1]
    ps = control_target[:, 2]
    t = control_target[:, 3]
    c, s = np.cos(ps / 2), np.sin(ps / 2)
    W, X, Y, Z = (q[:, i] for i in range(4))
    tw = c * W + s * Z
    tx = c * X + s * Y
    ty = c * Y - s * X
    tz = c * Z - s * W
    c, s = np.cos(ph / 2), np.sin(ph / 2)
    uw = c * tw + s * tx
    ux = c * tx - s * tw
    uy = c * ty + s * tz
    uz = c * tz - s * ty
    c, s = np.cos(th / 2), np.sin(th / 2)
    A = c * uw + s * uy
    Bq = c * ux - s * uz
    Cq = c * uy - s * uw
    D = c * uz + s * ux
    AB, AC, BD, CD = A * Bq, A * Cq, Bq * D, Cq * D
    M02 = BD + AC
    M12 = CD - AB
    Sg = Bq * Bq + Cq * Cq
    psw = ps * np.float32(2.0 * fp["wr"])
    psw2a = ps * np.float32(fp["wr2"])
    psw2b = ps * np.float32(2.0 * fp["wr2"])
    G0 = (2.0 * fp["wa"]) * AB + fp["wr"] * av[:, 0] - psw * M02
    G1 = (2.0 * fp["wa1"]) * AC + fp["wr1"] * av[:, 1] - psw * M12
    G2 = (fp["wr2"] * av[:, 2] - psw2a) + psw2b * Sg
    G3 = fp["wt"] * t - 1.0
    out = np.empty((root_state.shape[0], 4), np.float32)
    for r in range(4):
        out[:, r] = fp["sA"][r] * G0 + fp["sB"][r] * G1 + fp["sC"][r] * G2 + G3
    return out


# --------------------------------------------------------------------------
# Bass program builder (v3: pipelined front/body/back)
# --------------------------------------------------------------------------
def _front(nc, mybir, io, tmp, rs2, ct2, ti, c0, Cw, fp, cdt):
    f32 = mybir.dt.float32
    AF = mybir.ActivationFunctionType
    OP = mybir.AluOpType

    st = {"c0": c0, "Cw": Cw}
    rs_t = io.tile([P, Cw * 13], f32, tag="rs", name=f"rs_{ti}")
    nc.sync.dma_start(out=rs_t[:], in_=rs2[:, c0 * 13 : (c0 + Cw) * 13])
    ct_t = io.tile([P, Cw * 4], f32, tag="ct", name=f"ct_{ti}")
    nc.sync.dma_start(out=ct_t[:], in_=ct2[:, c0 * 4 : (c0 + Cw) * 4])
    rs3 = rs_t.rearrange("p (c m) -> p c m", m=13)
    ct3 = ct_t.rearrange("p (c m) -> p c m", m=4)
    st["ct3"] = ct3

    def tt(name, k=1):
        ap = tmp.tile([P, k * Cw], cdt, tag=name, name=f"{name}_{ti}")
        st[name] = ap
        return ap

    def v(ap, k):
        return ap.rearrange("p (k c) -> p k c", c=Cw)

    # ---- ACT: sin/cos triples over (roll, pitch, yaw)/2 ----
    ctT = ct3[:, :, 0:3].rearrange("p c m -> p m c")
    csn = tt("csn", 3)
    nc.scalar.activation(v(csn, 3), ctT, AF.Sin, bias=_PIO2, scale=0.5)
    ssn = tt("ssn", 3)
    nc.scalar.activation(v(ssn, 3), ctT, AF.Sin, bias=0.0, scale=0.5)
    # ---- ACT: yaw-rate scalings + thrust group ----
    psw = tt("psw")
    nc.scalar.activation(psw[:], ct3[:, :, 2], AF.Copy, scale=2.0 * fp["wr"])
    psw2b = tt("psw2b")
    nc.scalar.activation(psw2b[:], psw[:], AF.Copy, scale=fp["wr2"] / fp["wr"])
    GB = tt("GB", 4)
    nc.scalar.activation(v(GB, 4)[:, 0], ct3[:, :, 3], AF.Copy,
                         scale=fp["wt"], bias=-1.0)

    # ---- Pool: strided f32 extractions ----
    q4 = tt("q4", 4)
    rsT34 = rs3[:, :, 3:7].rearrange("p c m -> p m c")
    nc.gpsimd.tensor_scalar(v(q4, 4), rsT34, 1.0, None, OP.mult)
    eav01 = tt("eav01", 2)
    rsT01 = rs3[:, :, 10:12].rearrange("p c m -> p m c")
    nc.gpsimd.tensor_scalar(v(eav01, 2), rsT01, fp["wr"], None, OP.mult)
    psw2a = tt("psw2a")
    nc.gpsimd.tensor_scalar(psw2a[:], ct3[:, :, 2], fp["wr2"], None, OP.mult)
    eav2 = tt("eav2")
    nc.gpsimd.tensor_scalar(eav2[:], rs3[:, :, 12], fp["wr2"], None, OP.mult)
    g2a = tt("g2a")
    nc.gpsimd.tensor_tensor(g2a[:], eav2[:], psw2a[:], OP.subtract)
    return st


def _body(nc, mybir, tmp, st, ti, Cw, fp, cdt):
    OP = mybir.AluOpType
    TT = nc.vector.tensor_tensor

    def tt(name, k=1):
        ap = tmp.tile([P, k * Cw], cdt, tag=name, name=f"{name}_{ti}")
        st[name] = ap
        return ap

    def v(ap, k=None):
        if k is None:
            k = ap.shape[1] // Cw
        return ap.rearrange("p (k c) -> p k c", c=Cw)

    def bc(ap_pc, k):
        return (ap_pc.rearrange("p (k c) -> p k c", k=1)
                .to_broadcast([P, k, Cw]))

    def bc4d(ap_pc):
        return (ap_pc.rearrange("p (a b c) -> p a b c", a=1, b=1)
                .to_broadcast([P, 2, 2, Cw]))

    csn, ssn = v(st["csn"], 3), v(st["ssn"], 3)
    q4v = v(st["q4"], 4)

    # ---- stage 1: q_z(ps/2)* x q ----
    mc = tt("mc", 4)
    ms = tt("ms", 4)
    mcv, msv = v(mc, 4), v(ms, 4)
    TT(mcv[:, :], bc(csn[:, 2], 4), q4v[:, :], OP.mult)
    TT(msv[:, :], bc(ssn[:, 2], 4), q4v[:, ::-1], OP.mult)
    t4 = tt("t4", 4)
    t4v = v(t4, 4)
    TT(t4v[:, 0:2], mcv[:, 0:2], msv[:, 0:2], OP.add)
    TT(t4v[:, 2:4], mcv[:, 2:4], msv[:, 2:4], OP.subtract)

    # ---- stage 2: q_x(ph/2)* x t  (swap within pairs) ----
    TT(mcv[:, :], bc(csn[:, 0], 4), t4v[:, :], OP.mult)
    ms4d = st["ms"].rearrange("p (a b c) -> p a b c", a=2, c=Cw)
    t4sw = st["t4"].rearrange("p (a b c) -> p a b c", a=2, c=Cw)[:, :, ::-1]
    TT(ms4d, bc4d(ssn[:, 0]), t4sw, OP.mult)
    u4 = tt("u4", 4)
    u4v = v(u4, 4)
    TT(u4v[:, 0:4:2], mcv[:, 0:4:2], msv[:, 0:4:2], OP.add)
    TT(u4v[:, 1:4:2], mcv[:, 1:4:2], msv[:, 1:4:2], OP.subtract)

    # ---- stage 3: q_y(th/2)* x u  (rotate-2) ----
    TT(mcv[:, :], bc(csn[:, 1], 4), u4v[:, :], OP.mult)
    ms4r = st["ms"].rearrange("p (a b c) -> p a b c", b=2, c=Cw)
    u4rot = st["u4"].rearrange("p (a b c) -> p a b c", b=2, c=Cw)[:, ::-1]
    TT(ms4r, bc4d(ssn[:, 1]), u4rot, OP.mult)
    a4 = tt("a4", 4)
    a4v = v(a4, 4)
    TT(a4v[:, 0:4:3], mcv[:, 0:4:3], msv[:, 0:4:3], OP.add)
    TT(a4v[:, 1:3], mcv[:, 1:3], msv[:, 1:3], OP.subtract)

    # ---- products: P6 = (ab, ac, bd, cd, bb, cc) (UNdoubled) ----
    P6 = tt("P6", 6)
    P6v = v(P6, 6)
    TT(P6v[:, 0:2], bc(a4v[:, 0], 2), a4v[:, 1:3], OP.mult)
    TT(P6v[:, 2:4], a4v[:, 1:3], bc(a4v[:, 3], 2), OP.mult)
    TT(P6v[:, 4:6], a4v[:, 1:3], a4v[:, 1:3], OP.mult)

    # ---- M2 = (bd+ac, cd-ab), Sg = bb+cc ----
    M2 = tt("M2", 2)
    M2v = v(M2, 2)
    TT(M2v[:, 0], P6v[:, 2], P6v[:, 1], OP.add)
    TT(M2v[:, 1], P6v[:, 3], P6v[:, 0], OP.subtract)
    Sg = tt("Sg")
    TT(Sg[:], P6v[:, 4], P6v[:, 5], OP.add)

    # ---- s-terms ----
    s01 = tt("s01", 2)
    TT(v(s01, 2)[:, :], bc(st["psw"][:], 2), M2v[:, :], OP.mult)
    s2 = tt("s2")
    TT(s2[:], st["psw2b"][:], Sg[:], OP.mult)

    # ---- e13 (tensor_scalar hits DVE 4x mode), t01, GB, UV ----
    e13 = tt("e13", 2)
    nc.vector.tensor_scalar(e13[:], P6[:, 0 : 2 * Cw], 2.0 * fp["wa"], None,
                            OP.mult)
    t01 = tt("t01", 2)
    TT(v(t01, 2)[:, :], v(e13, 2)[:, :], v(st["eav01"], 2)[:, :], OP.add)
    GBv = v(st["GB"], 4)
    TT(GBv[:, 2:0:-1], v(t01, 2)[:, :], v(s01, 2)[:, :], OP.subtract)
    TT(GBv[:, 3], st["g2a"][:], s2[:], OP.add)
    UV = tt("UV", 4)
    UVv = v(UV, 4)
    TT(UVv[:, 0:2], GBv[:, 0:2], GBv[:, 2:4], OP.add)
    TT(UVv[:, 2:4], GBv[:, 0:2], GBv[:, 2:4], OP.subtract)


def _back(nc, mybir, io, st, ti, out2, fp, Cw):
    f32 = mybir.dt.float32
    OP = mybir.AluOpType
    c0 = st["c0"]

    out_t = io.tile([P, Cw * 4], f32, tag="out", name=f"out_{ti}")
    out3 = out_t.rearrange("p (c m) -> p c m", m=4)
    UVv = st["UV"].rearrange("p (k c) -> p k c", c=Cw)

    uidx = [0 if fp["sA"][r] > 0 else 2 for r in range(4)]
    vidx = [1 if fp["sB"][r] * fp["sC"][r] > 0 else 3 for r in range(4)]
    for r in range(4):
        op = OP.add if fp["sB"][r] > 0 else OP.subtract
        nc.gpsimd.tensor_tensor(out3[:, :, r], UVv[:, uidx[r]], UVv[:, vidx[r]],
                                op)
    nc.sync.dma_start(out=out2[:, c0 * 4 : (c0 + Cw) * 4], in_=out_t[:])


def _build_nc(fp, reps=1, trace_sim=False, cols=None, tile_widths=None):
    import concourse.bass as bass
    import concourse.mybir as mybir
    from concourse.tile import TileContext

    f32 = mybir.dt.float32
    cdt = getattr(mybir.dt, COMPUTE_DT)
    ncols = COLS if cols is None else cols
    widths = TILE_WIDTHS if tile_widths is None else tile_widths

    nc = bass.Bass()

    # const AP for the pi/2 bias used by cos-via-sin
    cbias = nc.alloc_sbuf_tensor("const-f32-pio2", [128, 1], f32)
    nc.gpsimd.memset(cbias.ap(), _PIO2)
    nc.const_aps.aps[(f32, _PIO2)] = cbias.ap()
    nc.all_engine_barrier()

    shard = ncols * P
    rs = nc.declare_dram_parameter("root_state", [shard, 13], f32, isOutput=False)
    ct = nc.declare_dram_parameter("control_target", [shard, 4], f32, isOutput=False)
    out = nc.declare_dram_parameter("out", [shard, 4], f32, isOutput=True)
    rs2 = rs.rearrange("(p c) m -> p (c m)", p=P)
    ct2 = ct.rearrange("(p c) m -> p (c m)", p=P)
    out2 = out.rearrange("(p c) m -> p (c m)", p=P)

    assert PARTIAL_OK or sum(widths) == ncols

    # flat schedule of (tile_index, col0, width) across reps
    tiles = []
    for rep in range(reps):
        c0 = 0
        for Cw in widths:
            tiles.append((len(tiles), c0, Cw))
            c0 += Cw

    with TileContext(nc, trace_sim=trace_sim) as tc:
        with (
            tc.tile_pool(name="io", bufs=IO_BUFS) as io,
            tc.tile_pool(name="tmp", bufs=TMP_BUFS) as tmp,
        ):
            sts = {}
            n = len(tiles)
            for j in range(min(LOOKAHEAD, n)):
                ti, c0, Cw = tiles[j]
                sts[ti] = _front(nc, mybir, io, tmp, rs2, ct2, ti, c0, Cw, fp, cdt)
            for i in range(n):
                ti, c0, Cw = tiles[i]
                _body(nc, mybir, tmp, sts[ti], ti, Cw, fp, cdt)
                _back(nc, mybir, io, sts[ti], ti, out2, fp, Cw)
                del sts[ti]
                if i + LOOKAHEAD < n:
                    tj, cj, Cwj = tiles[i + LOOKAHEAD]
                    sts[tj] = _front(nc, mybir, io, tmp, rs2, ct2, tj, cj, Cwj,
                                     fp, cdt)
    return nc


# --------------------------------------------------------------------------
# Public entry point
# --------------------------------------------------------------------------
def kernel(root_state, control_target, mass, g, mixer, max_thrusts,
           gain_attitude, gain_angular_rate):
    root_state = np.ascontiguousarray(np.asarray(root_state, np.float32))
    control_target = np.ascontiguousarray(np.asarray(control_target, np.float32))
    assert root_state.shape == (B_TOTAL, 13), root_state.shape
    assert control_target.shape == (B_TOTAL, 4), control_target.shape

    fp = _fold_params(mass, g, mixer, max_thrusts, gain_attitude, gain_angular_rate)

    key = hashlib.sha256(
        repr(({k: v for k, v in fp.items() if k != "Wf"}, COMPUTE_DT,
              tuple(TILE_WIDTHS), IO_BUFS, TMP_BUFS, LOOKAHEAD, "v3")).encode()
    ).hexdigest()
    if key not in _CACHE:
        _install_bir_patch()
        _CACHE[key] = _build_nc(fp)
    nc = _CACHE[key]

    from concourse.bass_utils import run_bass_kernel_spmd

    rs_shards = root_state.reshape(N_CORES, SHARD, 13)
    ct_shards = control_target.reshape(N_CORES, SHARD, 4)
    in_maps = [
        {"root_state": rs_shards[i], "control_target": ct_shards[i]}
        for i in range(N_CORES)
    ]
    res = run_bass_kernel_spmd(nc, in_maps, core_ids=list(range(N_CORES)))
    return np.concatenate([res.results[i]["out"] for i in range(N_CORES)], axis=0)


# revision 6
# speedup vs baseline: 2.5427x; 2.5427x over previous
"""Trainium2 Bass kernel for nn_AttitudeController (B=2097152 drones).

Contract: kernel(**inputs) takes the FULL unsharded inputs (numpy) and
returns the FULL [B, 4] float32 output.  Internally the batch is sharded
across 8 NeuronCores; each core runs an identical NEFF on its shard.

Math (derived from the reference):
    R_des^T R = R(q_err),  q_err = q_y(th/2)* x q_x(ph/2)* x q_z(ps/2)* x q
    angle_error = [2ab, 2ac, 0]          (a,b,c,d = q_err components)
    M[:,2]      = [2(bd+ac), 2(cd-ab), 1-2(b^2+c^2)]
    rate_error  = ang_vel - yaw_rate * M[:,2]
    out[r] = sum_k Wf[r,k] * f_k - 1
Wf has +-uniform-magnitude columns for the quad-X mixer, so the final
stage folds into 4 group values G0..G3 and a sign butterfly.  Products
stay UNdoubled on-device; the factors of 2 are folded into constants.

v3 structure (this file): software-pipelined emission.  Each tile is
split into front (DMA + ACT sines + GpSimd extractions), body (the DVE
quaternion chain), and back (GpSimd output butterfly + DMA out).  Fronts
run two tiles ahead so no engine ever waits on a same-tile round trip:
  emit: F0 F1 | B0 K0 F2 | B1 K1 F3 | B2 K2 | B3 K3
Engine budget per 512-col tile (measured rates):
  ACT  ~8.3us: 2 sin-triples, psw, psw2b, G3
  Pool ~9.8us: q4/eav01/psw2a extractions, g2a, 4 output writes
  DVE ~16.9us: 3 quat stages, 6 products, M/Sg, s-terms, GB, UV
  DMA ~16.6us: rs in, ct in, out
"""

import hashlib
import math

import numpy as np

B_TOTAL = 2097152
N_CORES = 8
SHARD = B_TOTAL // N_CORES          # 262144 rows per core
P = 128                             # SBUF partitions
COLS = SHARD // P                   # 2048 columns per partition

# --- tunables -------------------------------------------------------------
COMPUTE_DT = "float16"              # intermediate dtype on-chip
TILE_WIDTHS = [512, 512, 512, 512]  # column tiling of the 2048 cols
IO_BUFS = 2
TMP_BUFS = 2
LOOKAHEAD = 2                       # how many tiles the front runs ahead
PARTIAL_OK = False
MAX_WAITS = 1                       # walrus (this build) allows 1 wait/inst

_PIO2 = float(np.float32(math.pi / 2.0))
_SQRT2 = float(np.float32(math.sqrt(2.0)))   # used by folded_numpy only

_CACHE = {}


# --------------------------------------------------------------------------
# BIR post-processing: this walrus build rejects >1 sync-wait per
# instruction; split offenders into preceding Drain instructions.
# --------------------------------------------------------------------------
_bir_patch_installed = False


def _split_waits_in_bir(bir_bytes):
    import orjson

    d = orjson.loads(bir_bytes)
    changed = False
    mods = d.get("modules", [d]) if "functions" not in d else [d]
    for mod in mods:
        for fn in mod.get("functions", []):
            for blk in fn.get("blocks", []):
                out = []
                for ins in blk.get("instructions", []):
                    si = ins.get("sync_info") or {}
                    waits = si.get("on_wait") or []
                    if len(waits) > MAX_WAITS:
                        changed = True
                        chunks = [
                            waits[i : i + MAX_WAITS]
                            for i in range(0, len(waits), MAX_WAITS)
                        ]
                        for k, ch in enumerate(chunks[:-1]):
                            pre = {
                                "name": f"{ins['name']}-wsplit{k}",
                                "opcode": "Drain",
                                "engine": ins.get("engine", "SP"),
                                "ins": [],
                                "outs": [],
                                "is_reset_sema": False,
                                "sync_info": {"on_update": [], "on_wait": ch},
                            }
                            if "debug" in ins:
                                pre["debug"] = ins["debug"]
                            out.append(pre)
                        si["on_wait"] = chunks[-1]
                        ins["sync_info"] = si
                    out.append(ins)
                blk["instructions"] = out
    if changed:
        return orjson.dumps(d)
    return bir_bytes


def _install_bir_patch():
    global _bir_patch_installed
    if _bir_patch_installed:
        return
    from concourse import bass_utils

    orig = bass_utils.compile_bir_kernel

    def patched(bir_json, tmpdir, neff_name="file.neff", **kw):
        bj = bir_json if isinstance(bir_json, (bytes, bytearray)) else bir_json.encode()
        return orig(_split_waits_in_bir(bytes(bj)), tmpdir, neff_name=neff_name, **kw)

    bass_utils.compile_bir_kernel = patched
    # bass2jax imported the symbol directly
    from concourse import bass2jax

    bass2jax.compile_bir_kernel = patched
    _bir_patch_installed = True


# --------------------------------------------------------------------------
# Parameter folding
# --------------------------------------------------------------------------
def _fold_params(mass, g, mixer, max_thrusts, gain_attitude, gain_angular_rate):
    mixer = np.asarray(mixer, np.float64)
    mt = np.asarray(max_thrusts, np.float64)
    ga = np.asarray(gain_attitude, np.float64)
    gar = np.asarray(gain_angular_rate, np.float64)
    m2 = 2.0 * mixer / mt[:, None]  # [4 rotors, 4]
    Wf = np.zeros((4, 6))
    Wf[:, 0] = -m2[:, 0] * ga[0]     # coeff of 2ab
    Wf[:, 1] = -m2[:, 1] * ga[1]     # coeff of 2ac
    Wf[:, 2] = -m2[:, 0] * gar[0]    # coeff of rate_err0
    Wf[:, 3] = -m2[:, 1] * gar[1]    # coeff of rate_err1
    Wf[:, 4] = -m2[:, 2] * gar[2]    # coeff of rate_err2
    Wf[:, 5] = m2[:, 3] * float(mass) * float(g)

    def col_mag(k):
        m = np.abs(Wf[:, k])
        if not np.allclose(m, m[0], rtol=1e-5):
            raise RuntimeError(f"mixer column {k} magnitudes not uniform: {m}")
        return float(m[0])

    wa, wa1, wr, wr1, wr2, wt = (col_mag(k) for k in range(6))
    if not (np.isclose(wa, wa1, rtol=1e-5) and np.isclose(wr, wr1, rtol=1e-5)):
        raise RuntimeError("roll/pitch gain magnitudes differ; v3 needs wa==wa1")
    sA = np.sign(Wf[:, 0]).astype(int)
    sB = np.sign(Wf[:, 1]).astype(int)
    sC = np.sign(Wf[:, 4]).astype(int)
    if not (np.sign(Wf[:, 2]) == sA).all():
        raise RuntimeError("columns 0/2 sign mismatch")
    if not (np.sign(Wf[:, 3]) == sB).all():
        raise RuntimeError("columns 1/3 sign mismatch")
    if not (np.sign(Wf[:, 5]) > 0).all():
        raise RuntimeError("thrust column must be positive")
    return dict(
        wa=wa, wa1=wa1, wr=wr, wr1=wr1, wr2=wr2, wt=wt,
        sA=sA.tolist(), sB=sB.tolist(), sC=sC.tolist(), Wf=Wf,
    )


def folded_numpy(root_state, control_target, fp):
    """Numpy model of exactly what the device computes (fp32). Used by
    test.py to validate the algebra separately from the hardware."""
    q = root_state[:, 3:7].astype(np.float32)
    av = root_state[:, 10:13].astype(np.float32)
    ph = control_target[:, 0]
    th = control_target[:, 1]
    ps = control_target[:, 2]
    t = control_target[:, 3]
    c, s = np.cos(ps / 2), np.sin(ps / 2)
    W, X, Y, Z = (q[:, i] for i in range(4))
    tw = c * W + s * Z
    tx = c * X + s * Y
    ty = c * Y - s * X
    tz = c * Z - s * W
    c, s = np.cos(ph / 2), np.sin(ph / 2)
    uw = c * tw + s * tx
    ux = c * tx - s * tw
    uy = c * ty + s * tz
    uz = c * tz - s * ty
    c, s = np.cos(th / 2), np.sin(th / 2)
    A = c * uw + s * uy
    Bq = c * ux - s * uz
    Cq = c * uy - s * uw
    D = c * uz + s * ux
    AB, AC, BD, CD = A * Bq, A * Cq, Bq * D, Cq * D
    M02 = BD + AC
    M12 = CD - AB
    Sg = Bq * Bq + Cq * Cq
    psw = ps * np.float32(2.0 * fp["wr"])
    psw2a = ps * np.float32(fp["wr2"])
    psw2b = ps * np.float32(2.0 * fp["wr2"])
    G0 = (2.0 * fp["wa"]) * AB + fp["wr"] * av[:, 0] - psw * M02
    G1 = (2.0 * fp["wa1"]) * AC + fp["wr1"] * av[:, 1] - psw * M12
    G2 = (fp["wr2"] * av[:, 2] - psw2a) + psw2b * Sg
    G3 = fp["wt"] * t - 1.0
    out = np.empty((root_state.shape[0], 4), np.float32)
    for r in range(4):
        out[:, r] = fp["sA"][r] * G0 + fp["sB"][r] * G1 + fp["sC"][r] * G2 + G3
    return out


# --------------------------------------------------------------------------
# Bass program builder (v3: pipelined front/body/back)
# --------------------------------------------------------------------------
def _front(nc, mybir, io, tmp, rs2, ct2, ti, c0, Cw, fp, cdt):
    f32 = mybir.dt.float32
    AF = mybir.ActivationFunctionType
    OP = mybir.AluOpType

    st = {"c0": c0, "Cw": Cw}
    rs_t = io.tile([P, Cw * 13], f32, tag="rs", name=f"rs_{ti}")
    nc.sync.dma_start(out=rs_t[:], in_=rs2[:, c0 * 13 : (c0 + Cw) * 13])
    ct_t = io.tile([P, Cw * 4], f32, tag="ct", name=f"ct_{ti}")
    nc.sync.dma_start(out=ct_t[:], in_=ct2[:, c0 * 4 : (c0 + Cw) * 4])
    rs3 = rs_t.rearrange("p (c m) -> p c m", m=13)
    ct3 = ct_t.rearrange("p (c m) -> p c m", m=4)
    st["ct3"] = ct3

    def tt(name, k=1):
        ap = tmp.tile([P, k * Cw], cdt, tag=name, name=f"{name}_{ti}")
        st[name] = ap
        return ap

    def v(ap, k):
        return ap.rearrange("p (k c) -> p k c", c=Cw)

    # ---- ACT: sin/cos triples over (roll, pitch, yaw)/2 ----
    ctT = ct3[:, :, 0:3].rearrange("p c m -> p m c")
    csn = tt("csn", 3)
    nc.scalar.activation(v(csn, 3), ctT, AF.Sin, bias=_PIO2, scale=0.5)
    ssn = tt("ssn", 3)
    nc.scalar.activation(v(ssn, 3), ctT, AF.Sin, bias=0.0, scale=0.5)
    # ---- ACT: f32 -> fp16 extractions (ACT is src-byte-bound; one op each) ----
    q4 = tt("q4", 4)
    rsT34 = rs3[:, :, 3:7].rearrange("p c m -> p m c")
    nc.scalar.activation(v(q4, 4), rsT34, AF.Copy)
    av3 = tt("av3", 3)
    rsTav = rs3[:, :, 10:13].rearrange("p c m -> p m c")
    nc.scalar.activation(v(av3, 3), rsTav, AF.Copy)
    psw = tt("psw")
    nc.scalar.activation(psw[:], ct3[:, :, 2], AF.Copy, scale=2.0 * fp["wr"])
    GB = tt("GB", 4)
    nc.scalar.activation(v(GB, 4)[:, 0], ct3[:, :, 3], AF.Copy,
                         scale=fp["wt"], bias=-1.0)

    # ---- DVE tensor_scalar (4x perf mode on packed fp16) scalings ----
    TS = nc.vector.tensor_scalar
    eav01 = tt("eav01", 2)
    TS(eav01[:], av3[:, 0 : 2 * Cw], fp["wr"], None, OP.mult)
    eav2 = tt("eav2")
    TS(eav2[:], v(av3, 3)[:, 2], fp["wr2"], None, OP.mult)
    psw2a = tt("psw2a")
    TS(psw2a[:], psw[:], fp["wr2"] / (2.0 * fp["wr"]), None, OP.mult)
    psw2b = tt("psw2b")
    TS(psw2b[:], psw[:], fp["wr2"] / fp["wr"], None, OP.mult)

    # ---- Pool: g2a = wr2*av2 - wr2*ps (packed fp16 inputs) ----
    g2a = tt("g2a")
    nc.gpsimd.tensor_tensor(g2a[:], eav2[:], psw2a[:], OP.subtract)
    return st


def _body(nc, mybir, tmp, st, ti, Cw, fp, cdt):
    OP = mybir.AluOpType
    TT = nc.vector.tensor_tensor

    def tt(name, k=1, tag=None):
        ap = tmp.tile([P, k * Cw], cdt, tag=tag or name, name=f"{name}_{ti}")
        st[name] = ap
        return ap

    def v(ap, k=None):
        if k is None:
            k = ap.shape[1] // Cw
        return ap.rearrange("p (k c) -> p k c", c=Cw)

    def bc(ap_pc, k):
        return (ap_pc.rearrange("p (k c) -> p k c", k=1)
                .to_broadcast([P, k, Cw]))

    def bc4d(ap_pc):
        return (ap_pc.rearrange("p (a b c) -> p a b c", a=1, b=1)
                .to_broadcast([P, 2, 2, Cw]))

    csn, ssn = v(st["csn"], 3), v(st["ssn"], 3)
    q4v = v(st["q4"], 4)

    # ---- stage 1: q_z(ps/2)* x q ----
    mc = tt("mc", 4)
    ms = tt("ms", 4)
    mcv, msv = v(mc, 4), v(ms, 4)
    TT(mcv[:, :], bc(csn[:, 2], 4), q4v[:, :], OP.mult)
    TT(msv[:, :], bc(ssn[:, 2], 4), q4v[:, ::-1], OP.mult)
    t4 = tt("t4", 4, tag="st")
    t4v = v(t4, 4)
    TT(t4v[:, 0:2], mcv[:, 0:2], msv[:, 0:2], OP.add)
    TT(t4v[:, 2:4], mcv[:, 2:4], msv[:, 2:4], OP.subtract)

    # ---- stage 2: q_x(ph/2)* x t  (swap within pairs) ----
    TT(mcv[:, :], bc(csn[:, 0], 4), t4v[:, :], OP.mult)
    ms4d = st["ms"].rearrange("p (a b c) -> p a b c", a=2, c=Cw)
    t4sw = st["t4"].rearrange("p (a b c) -> p a b c", a=2, c=Cw)[:, :, ::-1]
    TT(ms4d, bc4d(ssn[:, 0]), t4sw, OP.mult)
    u4 = tt("u4", 4, tag="st")
    u4v = v(u4, 4)
    TT(u4v[:, 0:4:2], mcv[:, 0:4:2], msv[:, 0:4:2], OP.add)
    TT(u4v[:, 1:4:2], mcv[:, 1:4:2], msv[:, 1:4:2], OP.subtract)

    # ---- stage 3: q_y(th/2)* x u  (rotate-2) ----
    TT(mcv[:, :], bc(csn[:, 1], 4), u4v[:, :], OP.mult)
    ms4r = st["ms"].rearrange("p (a b c) -> p a b c", b=2, c=Cw)
    u4rot = st["u4"].rearrange("p (a b c) -> p a b c", b=2, c=Cw)[:, ::-1]
    TT(ms4r, bc4d(ssn[:, 1]), u4rot, OP.mult)
    a4 = tt("a4", 4, tag="st")
    a4v = v(a4, 4)
    TT(a4v[:, 0:4:3], mcv[:, 0:4:3], msv[:, 0:4:3], OP.add)
    TT(a4v[:, 1:3], mcv[:, 1:3], msv[:, 1:3], OP.subtract)

    # ---- products: P6 = (ab, ac, bd, cd, bb, cc) (UNdoubled) ----
    P6 = tt("P6", 6)
    P6v = v(P6, 6)
    TT(P6v[:, 0:2], bc(a4v[:, 0], 2), a4v[:, 1:3], OP.mult)
    TT(P6v[:, 2:4], a4v[:, 1:3], bc(a4v[:, 3], 2), OP.mult)
    TT(P6v[:, 4:6], a4v[:, 1:3], a4v[:, 1:3], OP.mult)

    # ---- M2 = (bd+ac, cd-ab), Sg = bb+cc ----
    M2 = tt("M2", 2)
    M2v = v(M2, 2)
    TT(M2v[:, 0], P6v[:, 2], P6v[:, 1], OP.add)
    TT(M2v[:, 1], P6v[:, 3], P6v[:, 0], OP.subtract)
    Sg = tt("Sg")
    TT(Sg[:], P6v[:, 4], P6v[:, 5], OP.add)

    # ---- s-terms ----
    s01 = tt("s01", 2)
    TT(v(s01, 2)[:, :], bc(st["psw"][:], 2), M2v[:, :], OP.mult)
    s2 = tt("s2")
    TT(s2[:], st["psw2b"][:], Sg[:], OP.mult)

    # ---- e13 (tensor_scalar hits DVE 4x mode), t01, GB, UV ----
    e13 = tt("e13", 2)
    nc.vector.tensor_scalar(e13[:], P6[:, 0 : 2 * Cw], 2.0 * fp["wa"], None,
                            OP.mult)
    t01 = tt("t01", 2)
    TT(v(t01, 2)[:, :], v(e13, 2)[:, :], v(st["eav01"], 2)[:, :], OP.add)
    GBv = v(st["GB"], 4)
    TT(GBv[:, 2:0:-1], v(t01, 2)[:, :], v(s01, 2)[:, :], OP.subtract)
    TT(GBv[:, 3], st["g2a"][:], s2[:], OP.add)
    UV = tt("UV", 4)
    UVv = v(UV, 4)
    TT(UVv[:, 0:2], GBv[:, 0:2], GBv[:, 2:4], OP.add)
    TT(UVv[:, 2:4], GBv[:, 0:2], GBv[:, 2:4], OP.subtract)


def _back(nc, mybir, io, st, ti, out2, fp, Cw):
    f32 = mybir.dt.float32
    OP = mybir.AluOpType
    c0 = st["c0"]

    out_t = io.tile([P, Cw * 4], f32, tag="out", name=f"out_{ti}")
    out3 = out_t.rearrange("p (c m) -> p c m", m=4)
    UVv = st["UV"].rearrange("p (k c) -> p k c", c=Cw)

    uidx = [0 if fp["sA"][r] > 0 else 2 for r in range(4)]
    vidx = [1 if fp["sB"][r] * fp["sC"][r] > 0 else 3 for r in range(4)]
    for r in range(4):
        op = OP.add if fp["sB"][r] > 0 else OP.subtract
        nc.gpsimd.tensor_tensor(out3[:, :, r], UVv[:, uidx[r]], UVv[:, vidx[r]],
                                op)
    nc.sync.dma_start(out=out2[:, c0 * 4 : (c0 + Cw) * 4], in_=out_t[:])


def _build_nc(fp, reps=1, trace_sim=False, cols=None, tile_widths=None):
    import concourse.bass as bass
    import concourse.mybir as mybir
    from concourse.tile import TileContext

    f32 = mybir.dt.float32
    cdt = getattr(mybir.dt, COMPUTE_DT)
    ncols = COLS if cols is None else cols
    widths = TILE_WIDTHS if tile_widths is None else tile_widths

    nc = bass.Bass()

    # const AP for the pi/2 bias used by cos-via-sin
    cbias = nc.alloc_sbuf_tensor("const-f32-pio2", [128, 1], f32)
    nc.gpsimd.memset(cbias.ap(), _PIO2)
    nc.const_aps.aps[(f32, _PIO2)] = cbias.ap()
    nc.all_engine_barrier()

    shard = ncols * P
    rs = nc.declare_dram_parameter("root_state", [shard, 13], f32, isOutput=False)
    ct = nc.declare_dram_parameter("control_target", [shard, 4], f32, isOutput=False)
    out = nc.declare_dram_parameter("out", [shard, 4], f32, isOutput=True)
    rs2 = rs.rearrange("(p c) m -> p (c m)", p=P)
    ct2 = ct.rearrange("(p c) m -> p (c m)", p=P)
    out2 = out.rearrange("(p c) m -> p (c m)", p=P)

    assert PARTIAL_OK or sum(widths) == ncols

    # flat schedule of (tile_index, col0, width) across reps
    tiles = []
    for rep in range(reps):
        c0 = 0
        for Cw in widths:
            tiles.append((len(tiles), c0, Cw))
            c0 += Cw

    with TileContext(nc, trace_sim=trace_sim) as tc:
        with (
            tc.tile_pool(name="io", bufs=IO_BUFS) as io,
            tc.tile_pool(name="tmp", bufs=TMP_BUFS) as tmp,
        ):
            sts = {}
            n = len(tiles)
            for j in range(min(LOOKAHEAD, n)):
                ti, c0, Cw = tiles[j]
                sts[ti] = _front(nc, mybir, io, tmp, rs2, ct2, ti, c0, Cw, fp, cdt)
            for i in range(n):
                ti, c0, Cw = tiles[i]
                _body(nc, mybir, tmp, sts[ti], ti, Cw, fp, cdt)
                _back(nc, mybir, io, sts[ti], ti, out2, fp, Cw)
                del sts[ti]
                if i + LOOKAHEAD < n:
                    tj, cj, Cwj = tiles[i + LOOKAHEAD]
                    sts[tj] = _front(nc, mybir, io, tmp, rs2, ct2, tj, cj, Cwj,
                                     fp, cdt)
    return nc


# --------------------------------------------------------------------------
# Public entry point
# --------------------------------------------------------------------------
def kernel(root_state, control_target, mass, g, mixer, max_thrusts,
           gain_attitude, gain_angular_rate):
    root_state = np.ascontiguousarray(np.asarray(root_state, np.float32))
    control_target = np.ascontiguousarray(np.asarray(control_target, np.float32))
    assert root_state.shape == (B_TOTAL, 13), root_state.shape
    assert control_target.shape == (B_TOTAL, 4), control_target.shape

    fp = _fold_params(mass, g, mixer, max_thrusts, gain_attitude, gain_angular_rate)

    key = hashlib.sha256(
        repr(({k: v for k, v in fp.items() if k != "Wf"}, COMPUTE_DT,
              tuple(TILE_WIDTHS), IO_BUFS, TMP_BUFS, LOOKAHEAD, "v4")).encode()
    ).hexdigest()
    if key not in _CACHE:
        _install_bir_patch()
        _CACHE[key] = _build_nc(fp)
    nc = _CACHE[key]

    from concourse.bass_utils import run_bass_kernel_spmd

    rs_shards = root_state.reshape(N_CORES, SHARD, 13)
    ct_shards = control_target.reshape(N_CORES, SHARD, 4)
    in_maps = [
        {"root_state": rs_shards[i], "control_target": ct_shards[i]}
        for i in range(N_CORES)
    ]
    res = run_bass_kernel_spmd(nc, in_maps, core_ids=list(range(N_CORES)))
    return np.concatenate([res.results[i]["out"] for i in range(N_CORES)], axis=0)


# revision 7
# speedup vs baseline: 2.6146x; 1.0283x over previous
"""Trainium2 Bass kernel for nn_AttitudeController (B=2097152 drones).

Contract: kernel(**inputs) takes the FULL unsharded inputs (numpy) and
returns the FULL [B, 4] float32 output.  Internally the batch is sharded
across 8 NeuronCores; each core runs an identical NEFF on its shard.

Math (derived from the reference):
    R_des^T R = R(q_err),  q_err = q_y(th/2)* x q_x(ph/2)* x q_z(ps/2)* x q
    angle_error = [2ab, 2ac, 0]          (a,b,c,d = q_err components)
    M[:,2]      = [2(bd+ac), 2(cd-ab), 1-2(b^2+c^2)]
    rate_error  = ang_vel - yaw_rate * M[:,2]
    out[r] = sum_k Wf[r,k] * f_k - 1
Wf has +-uniform-magnitude columns for the quad-X mixer, so the final
stage folds into 4 group values G0..G3 and a sign butterfly.  Products
stay UNdoubled on-device; the factors of 2 are folded into constants.

v3 structure (this file): software-pipelined emission.  Each tile is
split into front (DMA + ACT sines + GpSimd extractions), body (the DVE
quaternion chain), and back (GpSimd output butterfly + DMA out).  Fronts
run two tiles ahead so no engine ever waits on a same-tile round trip:
  emit: F0 F1 | B0 K0 F2 | B1 K1 F3 | B2 K2 | B3 K3
Engine budget per 512-col tile (measured rates):
  ACT  ~8.3us: 2 sin-triples, psw, psw2b, G3
  Pool ~9.8us: q4/eav01/psw2a extractions, g2a, 4 output writes
  DVE ~16.9us: 3 quat stages, 6 products, M/Sg, s-terms, GB, UV
  DMA ~16.6us: rs in, ct in, out
"""

import hashlib
import math

import numpy as np

B_TOTAL = 2097152
N_CORES = 8
SHARD = B_TOTAL // N_CORES          # 262144 rows per core
P = 128                             # SBUF partitions
COLS = SHARD // P                   # 2048 columns per partition

# --- tunables -------------------------------------------------------------
COMPUTE_DT = "float16"              # intermediate dtype on-chip
TILE_WIDTHS = [256, 448, 448, 448, 448]  # column tiling (small first tile = short fill)
IO_BUFS = 2
TMP_BUFS = 2
LOOKAHEAD = 1                       # how many tiles the front runs ahead
PARTIAL_OK = False
MAX_WAITS = 1                       # walrus (this build) allows 1 wait/inst

_PIO2 = float(np.float32(math.pi / 2.0))
_SQRT2 = float(np.float32(math.sqrt(2.0)))   # used by folded_numpy only

_CACHE = {}


# --------------------------------------------------------------------------
# BIR post-processing: this walrus build rejects >1 sync-wait per
# instruction; split offenders into preceding Drain instructions.
# --------------------------------------------------------------------------
_bir_patch_installed = False


def _split_waits_in_bir(bir_bytes):
    import orjson

    d = orjson.loads(bir_bytes)
    changed = False
    mods = d.get("modules", [d]) if "functions" not in d else [d]
    for mod in mods:
        for fn in mod.get("functions", []):
            for blk in fn.get("blocks", []):
                out = []
                for ins in blk.get("instructions", []):
                    si = ins.get("sync_info") or {}
                    waits = si.get("on_wait") or []
                    if len(waits) > MAX_WAITS:
                        changed = True
                        chunks = [
                            waits[i : i + MAX_WAITS]
                            for i in range(0, len(waits), MAX_WAITS)
                        ]
                        for k, ch in enumerate(chunks[:-1]):
                            pre = {
                                "name": f"{ins['name']}-wsplit{k}",
                                "opcode": "Drain",
                                "engine": ins.get("engine", "SP"),
                                "ins": [],
                                "outs": [],
                                "is_reset_sema": False,
                                "sync_info": {"on_update": [], "on_wait": ch},
                            }
                            if "debug" in ins:
                                pre["debug"] = ins["debug"]
                            out.append(pre)
                        si["on_wait"] = chunks[-1]
                        ins["sync_info"] = si
                    out.append(ins)
                blk["instructions"] = out
    if changed:
        return orjson.dumps(d)
    return bir_bytes


def _install_bir_patch():
    global _bir_patch_installed
    if _bir_patch_installed:
        return
    from concourse import bass_utils

    orig = bass_utils.compile_bir_kernel

    def patched(bir_json, tmpdir, neff_name="file.neff", **kw):
        bj = bir_json if isinstance(bir_json, (bytes, bytearray)) else bir_json.encode()
        return orig(_split_waits_in_bir(bytes(bj)), tmpdir, neff_name=neff_name, **kw)

    bass_utils.compile_bir_kernel = patched
    # bass2jax imported the symbol directly
    from concourse import bass2jax

    bass2jax.compile_bir_kernel = patched
    _bir_patch_installed = True


# --------------------------------------------------------------------------
# Parameter folding
# --------------------------------------------------------------------------
def _fold_params(mass, g, mixer, max_thrusts, gain_attitude, gain_angular_rate):
    mixer = np.asarray(mixer, np.float64)
    mt = np.asarray(max_thrusts, np.float64)
    ga = np.asarray(gain_attitude, np.float64)
    gar = np.asarray(gain_angular_rate, np.float64)
    m2 = 2.0 * mixer / mt[:, None]  # [4 rotors, 4]
    Wf = np.zeros((4, 6))
    Wf[:, 0] = -m2[:, 0] * ga[0]     # coeff of 2ab
    Wf[:, 1] = -m2[:, 1] * ga[1]     # coeff of 2ac
    Wf[:, 2] = -m2[:, 0] * gar[0]    # coeff of rate_err0
    Wf[:, 3] = -m2[:, 1] * gar[1]    # coeff of rate_err1
    Wf[:, 4] = -m2[:, 2] * gar[2]    # coeff of rate_err2
    Wf[:, 5] = m2[:, 3] * float(mass) * float(g)

    def col_mag(k):
        m = np.abs(Wf[:, k])
        if not np.allclose(m, m[0], rtol=1e-5):
            raise RuntimeError(f"mixer column {k} magnitudes not uniform: {m}")
        return float(m[0])

    wa, wa1, wr, wr1, wr2, wt = (col_mag(k) for k in range(6))
    if not (np.isclose(wa, wa1, rtol=1e-5) and np.isclose(wr, wr1, rtol=1e-5)):
        raise RuntimeError("roll/pitch gain magnitudes differ; v3 needs wa==wa1")
    sA = np.sign(Wf[:, 0]).astype(int)
    sB = np.sign(Wf[:, 1]).astype(int)
    sC = np.sign(Wf[:, 4]).astype(int)
    if not (np.sign(Wf[:, 2]) == sA).all():
        raise RuntimeError("columns 0/2 sign mismatch")
    if not (np.sign(Wf[:, 3]) == sB).all():
        raise RuntimeError("columns 1/3 sign mismatch")
    if not (np.sign(Wf[:, 5]) > 0).all():
        raise RuntimeError("thrust column must be positive")
    return dict(
        wa=wa, wa1=wa1, wr=wr, wr1=wr1, wr2=wr2, wt=wt,
        sA=sA.tolist(), sB=sB.tolist(), sC=sC.tolist(), Wf=Wf,
    )


def folded_numpy(root_state, control_target, fp):
    """Numpy model of exactly what the device computes (fp32). Used by
    test.py to validate the algebra separately from the hardware."""
    q = root_state[:, 3:7].astype(np.float32)
    av = root_state[:, 10:13].astype(np.float32)
    ph = control_target[:, 0]
    th = control_target[:, 1]
    ps = control_target[:, 2]
    t = control_target[:, 3]
    c, s = np.cos(ps / 2), np.sin(ps / 2)
    W, X, Y, Z = (q[:, i] for i in range(4))
    tw = c * W + s * Z
    tx = c * X + s * Y
    ty = c * Y - s * X
    tz = c * Z - s * W
    c, s = np.cos(ph / 2), np.sin(ph / 2)
    uw = c * tw + s * tx
    ux = c * tx - s * tw
    uy = c * ty + s * tz
    uz = c * tz - s * ty
    c, s = np.cos(th / 2), np.sin(th / 2)
    A = c * uw + s * uy
    Bq = c * ux - s * uz
    Cq = c * uy - s * uw
    D = c * uz + s * ux
    AB, AC, BD, CD = A * Bq, A * Cq, Bq * D, Cq * D
    M02 = BD + AC
    M12 = CD - AB
    Sg = Bq * Bq + Cq * Cq
    psw = ps * np.float32(2.0 * fp["wr"])
    psw2a = ps * np.float32(fp["wr2"])
    psw2b = ps * np.float32(2.0 * fp["wr2"])
    G0 = (2.0 * fp["wa"]) * AB + fp["wr"] * av[:, 0] - psw * M02
    G1 = (2.0 * fp["wa1"]) * AC + fp["wr1"] * av[:, 1] - psw * M12
    G2 = (fp["wr2"] * av[:, 2] - psw2a) + psw2b * Sg
    G3 = fp["wt"] * t - 1.0
    out = np.empty((root_state.shape[0], 4), np.float32)
    for r in range(4):
        out[:, r] = fp["sA"][r] * G0 + fp["sB"][r] * G1 + fp["sC"][r] * G2 + G3
    return out


# --------------------------------------------------------------------------
# Bass program builder (v3: pipelined front/body/back)
# --------------------------------------------------------------------------
def _front(nc, mybir, io, tmp, rs2, ct2, ti, c0, Cw, fp, cdt):
    f32 = mybir.dt.float32
    AF = mybir.ActivationFunctionType
    OP = mybir.AluOpType

    st = {"c0": c0, "Cw": Cw}
    rs_t = io.tile([P, Cw * 13], f32, tag="rs", name=f"rs_{ti}")
    nc.sync.dma_start(out=rs_t[:], in_=rs2[:, c0 * 13 : (c0 + Cw) * 13])
    ct_t = io.tile([P, Cw * 4], f32, tag="ct", name=f"ct_{ti}")
    nc.sync.dma_start(out=ct_t[:], in_=ct2[:, c0 * 4 : (c0 + Cw) * 4])
    rs3 = rs_t.rearrange("p (c m) -> p c m", m=13)
    ct3 = ct_t.rearrange("p (c m) -> p c m", m=4)
    st["ct3"] = ct3

    def tt(name, k=1):
        ap = tmp.tile([P, k * Cw], cdt, tag=name, name=f"{name}_{ti}")
        st[name] = ap
        return ap

    def v(ap, k):
        return ap.rearrange("p (k c) -> p k c", c=Cw)

    # ---- ACT: sin/cos triples over (roll, pitch, yaw)/2 ----
    ctT = ct3[:, :, 0:3].rearrange("p c m -> p m c")
    csn = tt("csn", 3)
    nc.scalar.activation(v(csn, 3), ctT, AF.Sin, bias=_PIO2, scale=0.5)
    ssn = tt("ssn", 3)
    nc.scalar.activation(v(ssn, 3), ctT, AF.Sin, bias=0.0, scale=0.5)
    # ---- ACT: f32 -> fp16 extractions (ACT is src-byte-bound; one op each).
    # Order matters: stage-1 inputs (csn/ssn/q4 above) first, the rest after.
    q4 = tt("q4", 4)
    rsT34 = rs3[:, :, 3:7].rearrange("p c m -> p m c")
    nc.scalar.activation(v(q4, 4), rsT34, AF.Copy)
    av3 = tt("av3", 3)
    rsTav = rs3[:, :, 10:13].rearrange("p c m -> p m c")
    nc.scalar.activation(v(av3, 3), rsTav, AF.Copy)
    psw = tt("psw")
    nc.scalar.activation(psw[:], ct3[:, :, 2], AF.Copy, scale=2.0 * fp["wr"])
    GB = tt("GB", 4)
    nc.scalar.activation(v(GB, 4)[:, 0], ct3[:, :, 3], AF.Copy,
                         scale=fp["wt"], bias=-1.0)

    # ---- ACT: psw2a/psw2b (fp16 src, cheap on ACT; keeps DVE free) ----
    psw2a = tt("psw2a")
    nc.scalar.activation(psw2a[:], psw[:], AF.Copy,
                         scale=fp["wr2"] / (2.0 * fp["wr"]))
    psw2b = tt("psw2b")
    nc.scalar.activation(psw2b[:], psw[:], AF.Copy, scale=fp["wr2"] / fp["wr"])
    return st


def _body(nc, mybir, tmp, st, ti, Cw, fp, cdt):
    OP = mybir.AluOpType
    TT = nc.vector.tensor_tensor

    def tt(name, k=1, tag=None):
        ap = tmp.tile([P, k * Cw], cdt, tag=tag or name, name=f"{name}_{ti}")
        st[name] = ap
        return ap

    def v(ap, k=None):
        if k is None:
            k = ap.shape[1] // Cw
        return ap.rearrange("p (k c) -> p k c", c=Cw)

    def bc(ap_pc, k):
        return (ap_pc.rearrange("p (k c) -> p k c", k=1)
                .to_broadcast([P, k, Cw]))

    def bc4d(ap_pc):
        return (ap_pc.rearrange("p (a b c) -> p a b c", a=1, b=1)
                .to_broadcast([P, 2, 2, Cw]))

    csn, ssn = v(st["csn"], 3), v(st["ssn"], 3)
    q4v = v(st["q4"], 4)
    TS = nc.vector.tensor_scalar

    # ---- stage 1: q_z(ps/2)* x q ----
    mc = tt("mc", 4)
    ms = tt("ms", 4)
    mcv, msv = v(mc, 4), v(ms, 4)
    TT(mcv[:, :], bc(csn[:, 2], 4), q4v[:, :], OP.mult)
    TT(msv[:, :], bc(ssn[:, 2], 4), q4v[:, ::-1], OP.mult)
    t4 = tt("t4", 4, tag="st")
    t4v = v(t4, 4)
    TT(t4v[:, 0:2], mcv[:, 0:2], msv[:, 0:2], OP.add)
    TT(t4v[:, 2:4], mcv[:, 2:4], msv[:, 2:4], OP.subtract)

    # ---- early TS extractions + Pool g2a (runs while DVE does stages) ----
    av3 = st["av3"]
    eav01 = tt("eav01", 2)
    TS(eav01[:], av3[:, 0 : 2 * Cw], fp["wr"], None, OP.mult)
    eav2 = tt("eav2")
    TS(eav2[:], v(av3)[:, 2], fp["wr2"], None, OP.mult)
    g2a = tt("g2a")
    nc.gpsimd.tensor_tensor(g2a[:], eav2[:], st["psw2a"][:], OP.subtract)

    # ---- stage 2: q_x(ph/2)* x t  (swap within pairs) ----
    TT(mcv[:, :], bc(csn[:, 0], 4), t4v[:, :], OP.mult)
    ms4d = st["ms"].rearrange("p (a b c) -> p a b c", a=2, c=Cw)
    t4sw = st["t4"].rearrange("p (a b c) -> p a b c", a=2, c=Cw)[:, :, ::-1]
    TT(ms4d, bc4d(ssn[:, 0]), t4sw, OP.mult)
    u4 = tt("u4", 4, tag="st")
    u4v = v(u4, 4)
    TT(u4v[:, 0:4:2], mcv[:, 0:4:2], msv[:, 0:4:2], OP.add)
    TT(u4v[:, 1:4:2], mcv[:, 1:4:2], msv[:, 1:4:2], OP.subtract)

    # ---- stage 3: q_y(th/2)* x u  (rotate-2) ----
    TT(mcv[:, :], bc(csn[:, 1], 4), u4v[:, :], OP.mult)
    ms4r = st["ms"].rearrange("p (a b c) -> p a b c", b=2, c=Cw)
    u4rot = st["u4"].rearrange("p (a b c) -> p a b c", b=2, c=Cw)[:, ::-1]
    TT(ms4r, bc4d(ssn[:, 1]), u4rot, OP.mult)
    a4 = tt("a4", 4, tag="st")
    a4v = v(a4, 4)
    TT(a4v[:, 0:4:3], mcv[:, 0:4:3], msv[:, 0:4:3], OP.add)
    TT(a4v[:, 1:3], mcv[:, 1:3], msv[:, 1:3], OP.subtract)

    # ---- products: P6 = (ab, ac, bd, cd, bb, cc) (UNdoubled) ----
    P6 = tt("P6", 6)
    P6v = v(P6, 6)
    TT(P6v[:, 0:2], bc(a4v[:, 0], 2), a4v[:, 1:3], OP.mult)
    TT(P6v[:, 2:4], a4v[:, 1:3], bc(a4v[:, 3], 2), OP.mult)
    TT(P6v[:, 4:6], a4v[:, 1:3], a4v[:, 1:3], OP.mult)

    # ---- M2 = (bd+ac, cd-ab), Sg = bb+cc ----
    M2 = tt("M2", 2)
    M2v = v(M2, 2)
    TT(M2v[:, 0], P6v[:, 2], P6v[:, 1], OP.add)
    TT(M2v[:, 1], P6v[:, 3], P6v[:, 0], OP.subtract)
    Sg = tt("Sg")
    TT(Sg[:], P6v[:, 4], P6v[:, 5], OP.add)

    # ---- s-terms ----
    s01 = tt("s01", 2)
    TT(v(s01, 2)[:, :], bc(st["psw"][:], 2), M2v[:, :], OP.mult)
    s2 = tt("s2")
    TT(s2[:], st["psw2b"][:], Sg[:], OP.mult)

    # ---- e13 (tensor_scalar hits DVE 4x mode), t01, GB, UV ----
    e13 = tt("e13", 2)
    nc.vector.tensor_scalar(e13[:], P6[:, 0 : 2 * Cw], 2.0 * fp["wa"], None,
                            OP.mult)
    t01 = tt("t01", 2)
    TT(v(t01, 2)[:, :], v(e13, 2)[:, :], v(st["eav01"], 2)[:, :], OP.add)
    GBv = v(st["GB"], 4)
    TT(GBv[:, 2:0:-1], v(t01, 2)[:, :], v(s01, 2)[:, :], OP.subtract)
    TT(GBv[:, 3], g2a[:], s2[:], OP.add)
    UV = tt("UV", 4)
    UVv = v(UV, 4)
    TT(UVv[:, 0:2], GBv[:, 0:2], GBv[:, 2:4], OP.add)
    TT(UVv[:, 2:4], GBv[:, 0:2], GBv[:, 2:4], OP.subtract)


def _back(nc, mybir, io, st, ti, out2, fp, Cw):
    f32 = mybir.dt.float32
    OP = mybir.AluOpType
    c0 = st["c0"]

    out_t = io.tile([P, Cw * 4], f32, tag="out", name=f"out_{ti}")
    out3 = out_t.rearrange("p (c m) -> p c m", m=4)
    UVv = st["UV"].rearrange("p (k c) -> p k c", c=Cw)

    uidx = [0 if fp["sA"][r] > 0 else 2 for r in range(4)]
    vidx = [1 if fp["sB"][r] * fp["sC"][r] > 0 else 3 for r in range(4)]
    for r in range(4):
        op = OP.add if fp["sB"][r] > 0 else OP.subtract
        nc.gpsimd.tensor_tensor(out3[:, :, r], UVv[:, uidx[r]], UVv[:, vidx[r]],
                                op)
    nc.sync.dma_start(out=out2[:, c0 * 4 : (c0 + Cw) * 4], in_=out_t[:])


def _build_nc(fp, reps=1, trace_sim=False, cols=None, tile_widths=None):
    import concourse.bass as bass
    import concourse.mybir as mybir
    from concourse.tile import TileContext

    f32 = mybir.dt.float32
    cdt = getattr(mybir.dt, COMPUTE_DT)
    ncols = COLS if cols is None else cols
    widths = TILE_WIDTHS if tile_widths is None else tile_widths

    nc = bass.Bass()

    # const AP for the pi/2 bias used by cos-via-sin
    cbias = nc.alloc_sbuf_tensor("const-f32-pio2", [128, 1], f32)
    nc.gpsimd.memset(cbias.ap(), _PIO2)
    nc.const_aps.aps[(f32, _PIO2)] = cbias.ap()
    nc.all_engine_barrier()

    shard = ncols * P
    rs = nc.declare_dram_parameter("root_state", [shard, 13], f32, isOutput=False)
    ct = nc.declare_dram_parameter("control_target", [shard, 4], f32, isOutput=False)
    out = nc.declare_dram_parameter("out", [shard, 4], f32, isOutput=True)
    rs2 = rs.rearrange("(p c) m -> p (c m)", p=P)
    ct2 = ct.rearrange("(p c) m -> p (c m)", p=P)
    out2 = out.rearrange("(p c) m -> p (c m)", p=P)

    assert PARTIAL_OK or sum(widths) == ncols

    # flat schedule of (tile_index, col0, width) across reps
    tiles = []
    for rep in range(reps):
        c0 = 0
        for Cw in widths:
            tiles.append((len(tiles), c0, Cw))
            c0 += Cw

    with TileContext(nc, trace_sim=trace_sim) as tc:
        with (
            tc.tile_pool(name="io", bufs=IO_BUFS) as io,
            tc.tile_pool(name="tmp", bufs=TMP_BUFS) as tmp,
        ):
            sts = {}
            n = len(tiles)
            for j in range(min(LOOKAHEAD, n)):
                ti, c0, Cw = tiles[j]
                sts[ti] = _front(nc, mybir, io, tmp, rs2, ct2, ti, c0, Cw, fp, cdt)
            for i in range(n):
                ti, c0, Cw = tiles[i]
                _body(nc, mybir, tmp, sts[ti], ti, Cw, fp, cdt)
                _back(nc, mybir, io, sts[ti], ti, out2, fp, Cw)
                del sts[ti]
                if i + LOOKAHEAD < n:
                    tj, cj, Cwj = tiles[i + LOOKAHEAD]
                    sts[tj] = _front(nc, mybir, io, tmp, rs2, ct2, tj, cj, Cwj,
                                     fp, cdt)
    return nc


# --------------------------------------------------------------------------
# Public entry point
# --------------------------------------------------------------------------
def kernel(root_state, control_target, mass, g, mixer, max_thrusts,
           gain_attitude, gain_angular_rate):
    root_state = np.ascontiguousarray(np.asarray(root_state, np.float32))
    control_target = np.ascontiguousarray(np.asarray(control_target, np.float32))
    assert root_state.shape == (B_TOTAL, 13), root_state.shape
    assert control_target.shape == (B_TOTAL, 4), control_target.shape

    fp = _fold_params(mass, g, mixer, max_thrusts, gain_attitude, gain_angular_rate)

    key = hashlib.sha256(
        repr(({k: v for k, v in fp.items() if k != "Wf"}, COMPUTE_DT,
              tuple(TILE_WIDTHS), IO_BUFS, TMP_BUFS, LOOKAHEAD, "v5")).encode()
    ).hexdigest()
    if key not in _CACHE:
        _install_bir_patch()
        _CACHE[key] = _build_nc(fp)
    nc = _CACHE[key]

    from concourse.bass_utils import run_bass_kernel_spmd

    rs_shards = root_state.reshape(N_CORES, SHARD, 13)
    ct_shards = control_target.reshape(N_CORES, SHARD, 4)
    in_maps = [
        {"root_state": rs_shards[i], "control_target": ct_shards[i]}
        for i in range(N_CORES)
    ]
    res = run_bass_kernel_spmd(nc, in_maps, core_ids=list(range(N_CORES)))
    return np.concatenate([res.results[i]["out"] for i in range(N_CORES)], axis=0)


# revision 8
# speedup vs baseline: 2.6545x; 1.0153x over previous
"""Trainium2 Bass kernel for nn_AttitudeController (B=2097152 drones).

Contract: kernel(**inputs) takes the FULL unsharded inputs (numpy) and
returns the FULL [B, 4] float32 output.  Internally the batch is sharded
across 8 NeuronCores; each core runs an identical NEFF on its shard.

Math (derived from the reference):
    R_des^T R = R(q_err),  q_err = q_y(th/2)* x q_x(ph/2)* x q_z(ps/2)* x q
    angle_error = [2ab, 2ac, 0]          (a,b,c,d = q_err components)
    M[:,2]      = [2(bd+ac), 2(cd-ab), 1-2(b^2+c^2)]
    rate_error  = ang_vel - yaw_rate * M[:,2]
    out[r] = sum_k Wf[r,k] * f_k - 1
Wf has +-uniform-magnitude columns for the quad-X mixer, so the final
stage folds into 4 group values G0..G3 and a sign butterfly.  Products
stay UNdoubled on-device; the factors of 2 are folded into constants.

v3 structure (this file): software-pipelined emission.  Each tile is
split into front (DMA + ACT sines + GpSimd extractions), body (the DVE
quaternion chain), and back (GpSimd output butterfly + DMA out).  Fronts
run two tiles ahead so no engine ever waits on a same-tile round trip:
  emit: F0 F1 | B0 K0 F2 | B1 K1 F3 | B2 K2 | B3 K3
Engine budget per 512-col tile (measured rates):
  ACT  ~8.3us: 2 sin-triples, psw, psw2b, G3
  Pool ~9.8us: q4/eav01/psw2a extractions, g2a, 4 output writes
  DVE ~16.9us: 3 quat stages, 6 products, M/Sg, s-terms, GB, UV
  DMA ~16.6us: rs in, ct in, out
"""

import hashlib
import math

import numpy as np

B_TOTAL = 2097152
N_CORES = 8
SHARD = B_TOTAL // N_CORES          # 262144 rows per core
P = 128                             # SBUF partitions
COLS = SHARD // P                   # 2048 columns per partition

# --- tunables -------------------------------------------------------------
COMPUTE_DT = "float16"              # intermediate dtype on-chip
TILE_WIDTHS = [256, 448, 448, 448, 448]  # column tiling (small first tile = short fill)
IO_BUFS = 2
TMP_BUFS = 2
LOOKAHEAD = 1                       # how many tiles the front runs ahead
PARTIAL_OK = False
MAX_WAITS = 1                       # walrus (this build) allows 1 wait/inst

_PIO2 = float(np.float32(math.pi / 2.0))
_SQRT2 = float(np.float32(math.sqrt(2.0)))   # used by folded_numpy only

_CACHE = {}


# --------------------------------------------------------------------------
# BIR post-processing: this walrus build rejects >1 sync-wait per
# instruction; split offenders into preceding Drain instructions.
# --------------------------------------------------------------------------
_bir_patch_installed = False


def _split_waits_in_bir(bir_bytes):
    import orjson

    d = orjson.loads(bir_bytes)
    changed = False
    mods = d.get("modules", [d]) if "functions" not in d else [d]
    for mod in mods:
        for fn in mod.get("functions", []):
            for blk in fn.get("blocks", []):
                out = []
                for ins in blk.get("instructions", []):
                    si = ins.get("sync_info") or {}
                    waits = si.get("on_wait") or []
                    if len(waits) > MAX_WAITS:
                        changed = True
                        chunks = [
                            waits[i : i + MAX_WAITS]
                            for i in range(0, len(waits), MAX_WAITS)
                        ]
                        for k, ch in enumerate(chunks[:-1]):
                            pre = {
                                "name": f"{ins['name']}-wsplit{k}",
                                "opcode": "Drain",
                                "engine": ins.get("engine", "SP"),
                                "ins": [],
                                "outs": [],
                                "is_reset_sema": False,
                                "sync_info": {"on_update": [], "on_wait": ch},
                            }
                            if "debug" in ins:
                                pre["debug"] = ins["debug"]
                            out.append(pre)
                        si["on_wait"] = chunks[-1]
                        ins["sync_info"] = si
                    out.append(ins)
                blk["instructions"] = out
    if changed:
        return orjson.dumps(d)
    return bir_bytes


def _install_bir_patch():
    global _bir_patch_installed
    if _bir_patch_installed:
        return
    from concourse import bass_utils

    orig = bass_utils.compile_bir_kernel

    def patched(bir_json, tmpdir, neff_name="file.neff", **kw):
        bj = bir_json if isinstance(bir_json, (bytes, bytearray)) else bir_json.encode()
        return orig(_split_waits_in_bir(bytes(bj)), tmpdir, neff_name=neff_name, **kw)

    bass_utils.compile_bir_kernel = patched
    # bass2jax imported the symbol directly
    from concourse import bass2jax

    bass2jax.compile_bir_kernel = patched
    _bir_patch_installed = True


# --------------------------------------------------------------------------
# Parameter folding
# --------------------------------------------------------------------------
def _fold_params(mass, g, mixer, max_thrusts, gain_attitude, gain_angular_rate):
    mixer = np.asarray(mixer, np.float64)
    mt = np.asarray(max_thrusts, np.float64)
    ga = np.asarray(gain_attitude, np.float64)
    gar = np.asarray(gain_angular_rate, np.float64)
    m2 = 2.0 * mixer / mt[:, None]  # [4 rotors, 4]
    Wf = np.zeros((4, 6))
    Wf[:, 0] = -m2[:, 0] * ga[0]     # coeff of 2ab
    Wf[:, 1] = -m2[:, 1] * ga[1]     # coeff of 2ac
    Wf[:, 2] = -m2[:, 0] * gar[0]    # coeff of rate_err0
    Wf[:, 3] = -m2[:, 1] * gar[1]    # coeff of rate_err1
    Wf[:, 4] = -m2[:, 2] * gar[2]    # coeff of rate_err2
    Wf[:, 5] = m2[:, 3] * float(mass) * float(g)

    def col_mag(k):
        m = np.abs(Wf[:, k])
        if not np.allclose(m, m[0], rtol=1e-5):
            raise RuntimeError(f"mixer column {k} magnitudes not uniform: {m}")
        return float(m[0])

    wa, wa1, wr, wr1, wr2, wt = (col_mag(k) for k in range(6))
    if not (np.isclose(wa, wa1, rtol=1e-5) and np.isclose(wr, wr1, rtol=1e-5)):
        raise RuntimeError("roll/pitch gain magnitudes differ; v3 needs wa==wa1")
    sA = np.sign(Wf[:, 0]).astype(int)
    sB = np.sign(Wf[:, 1]).astype(int)
    sC = np.sign(Wf[:, 4]).astype(int)
    if not (np.sign(Wf[:, 2]) == sA).all():
        raise RuntimeError("columns 0/2 sign mismatch")
    if not (np.sign(Wf[:, 3]) == sB).all():
        raise RuntimeError("columns 1/3 sign mismatch")
    if not (np.sign(Wf[:, 5]) > 0).all():
        raise RuntimeError("thrust column must be positive")
    return dict(
        wa=wa, wa1=wa1, wr=wr, wr1=wr1, wr2=wr2, wt=wt,
        sA=sA.tolist(), sB=sB.tolist(), sC=sC.tolist(), Wf=Wf,
    )


def folded_numpy(root_state, control_target, fp):
    """Numpy model of exactly what the device computes (fp32). Used by
    test.py to validate the algebra separately from the hardware."""
    q = root_state[:, 3:7].astype(np.float32)
    av = root_state[:, 10:13].astype(np.float32)
    ph = control_target[:, 0]
    th = control_target[:, 1]
    ps = control_target[:, 2]
    t = control_target[:, 3]
    c, s = np.cos(ps / 2), np.sin(ps / 2)
    W, X, Y, Z = (q[:, i] for i in range(4))
    tw = c * W + s * Z
    tx = c * X + s * Y
    ty = c * Y - s * X
    tz = c * Z - s * W
    c, s = np.cos(ph / 2), np.sin(ph / 2)
    uw = c * tw + s * tx
    ux = c * tx - s * tw
    uy = c * ty + s * tz
    uz = c * tz - s * ty
    c, s = np.cos(th / 2), np.sin(th / 2)
    A = c * uw + s * uy
    Bq = c * ux - s * uz
    Cq = c * uy - s * uw
    D = c * uz + s * ux
    AB, AC, BD, CD = A * Bq, A * Cq, Bq * D, Cq * D
    M02 = BD + AC
    M12 = CD - AB
    Sg = Bq * Bq + Cq * Cq
    psw = ps * np.float32(2.0 * fp["wr"])
    psw2a = ps * np.float32(fp["wr2"])
    psw2b = ps * np.float32(2.0 * fp["wr2"])
    G0 = (2.0 * fp["wa"]) * AB + fp["wr"] * av[:, 0] - psw * M02
    G1 = (2.0 * fp["wa1"]) * AC + fp["wr1"] * av[:, 1] - psw * M12
    G2 = (fp["wr2"] * av[:, 2] - psw2a) + psw2b * Sg
    G3 = fp["wt"] * t - 1.0
    out = np.empty((root_state.shape[0], 4), np.float32)
    for r in range(4):
        out[:, r] = fp["sA"][r] * G0 + fp["sB"][r] * G1 + fp["sC"][r] * G2 + G3
    return out


# --------------------------------------------------------------------------
# Bass program builder (v3: pipelined front/body/back)
# --------------------------------------------------------------------------
def _front(nc, mybir, io, tmp, rs2, ct2, ti, c0, Cw, fp, cdt):
    f32 = mybir.dt.float32
    AF = mybir.ActivationFunctionType
    OP = mybir.AluOpType

    st = {"c0": c0, "Cw": Cw}
    rs_t = io.tile([P, Cw * 13], f32, tag="rs", name=f"rs_{ti}")
    nc.sync.dma_start(out=rs_t[:], in_=rs2[:, c0 * 13 : (c0 + Cw) * 13])
    ct_t = io.tile([P, Cw * 4], f32, tag="ct", name=f"ct_{ti}")
    nc.sync.dma_start(out=ct_t[:], in_=ct2[:, c0 * 4 : (c0 + Cw) * 4])
    rs3 = rs_t.rearrange("p (c m) -> p c m", m=13)
    ct3 = ct_t.rearrange("p (c m) -> p c m", m=4)
    st["ct3"] = ct3

    def tt(name, k=1):
        ap = tmp.tile([P, k * Cw], cdt, tag=name, name=f"{name}_{ti}")
        st[name] = ap
        return ap

    def v(ap, k):
        return ap.rearrange("p (k c) -> p k c", c=Cw)

    # ---- ACT: sin/cos triples over (roll, pitch, yaw)/2 ----
    ctT = ct3[:, :, 0:3].rearrange("p c m -> p m c")
    csn = tt("csn", 3)
    nc.scalar.activation(v(csn, 3), ctT, AF.Sin, bias=_PIO2, scale=0.5)
    ssn = tt("ssn", 3)
    nc.scalar.activation(v(ssn, 3), ctT, AF.Sin, bias=0.0, scale=0.5)
    # ---- ACT: f32 -> fp16 extractions (ACT is src-byte-bound; one op each).
    # Order matters: stage-1 inputs (csn/ssn/q4 above) first, the rest after.
    q4 = tt("q4", 4)
    rsT34 = rs3[:, :, 3:7].rearrange("p c m -> p m c")
    nc.scalar.activation(v(q4, 4), rsT34, AF.Copy)
    av3 = tt("av3", 3)
    rsTav = rs3[:, :, 10:13].rearrange("p c m -> p m c")
    nc.scalar.activation(v(av3, 3), rsTav, AF.Copy)
    psw = tt("psw")
    nc.scalar.activation(psw[:], ct3[:, :, 2], AF.Copy, scale=2.0 * fp["wr"])
    GB = tt("GB", 4)
    nc.scalar.activation(v(GB, 4)[:, 0], ct3[:, :, 3], AF.Copy,
                         scale=fp["wt"], bias=-1.0)

    # ---- ACT: psw2a/psw2b (fp16 src, cheap on ACT; keeps DVE free) ----
    psw2a = tt("psw2a")
    nc.scalar.activation(psw2a[:], psw[:], AF.Copy,
                         scale=fp["wr2"] / (2.0 * fp["wr"]))
    psw2b = tt("psw2b")   # holds MINUS 2*wr2*ps (sign folded for s3/GB merge)
    nc.scalar.activation(psw2b[:], psw[:], AF.Copy, scale=-fp["wr2"] / fp["wr"])
    return st


def _body(nc, mybir, tmp, st, ti, Cw, fp, cdt):
    OP = mybir.AluOpType
    TT = nc.vector.tensor_tensor

    def tt(name, k=1, tag=None):
        ap = tmp.tile([P, k * Cw], cdt, tag=tag or name, name=f"{name}_{ti}")
        st[name] = ap
        return ap

    def v(ap, k=None):
        if k is None:
            k = ap.shape[1] // Cw
        return ap.rearrange("p (k c) -> p k c", c=Cw)

    def bc(ap_pc, k):
        return (ap_pc.rearrange("p (k c) -> p k c", k=1)
                .to_broadcast([P, k, Cw]))

    def bc4d(ap_pc):
        return (ap_pc.rearrange("p (a b c) -> p a b c", a=1, b=1)
                .to_broadcast([P, 2, 2, Cw]))

    csn, ssn = v(st["csn"], 3), v(st["ssn"], 3)
    q4v = v(st["q4"], 4)
    TS = nc.vector.tensor_scalar

    # ---- stage 1: q_z(ps/2)* x q ----
    mc = tt("mc", 4)
    ms = tt("ms", 4)
    mcv, msv = v(mc, 4), v(ms, 4)
    TT(mcv[:, :], bc(csn[:, 2], 4), q4v[:, :], OP.mult)
    TT(msv[:, :], bc(ssn[:, 2], 4), q4v[:, ::-1], OP.mult)
    t4 = tt("t4", 4, tag="st")
    t4v = v(t4, 4)
    TT(t4v[:, 0:2], mcv[:, 0:2], msv[:, 0:2], OP.add)
    TT(t4v[:, 2:4], mcv[:, 2:4], msv[:, 2:4], OP.subtract)

    # ---- early TS extractions + Pool g2a (runs while DVE does stages) ----
    av3 = st["av3"]
    eav01 = tt("eav01", 2)
    TS(eav01[:], av3[:, 0 : 2 * Cw], fp["wr"], None, OP.mult)
    eav2 = tt("eav2")
    TS(eav2[:], v(av3)[:, 2], fp["wr2"], None, OP.mult)
    T3 = tt("T3", 3)   # (G1-part, G0-part, g2a); comp2 written by Pool
    T3v = v(T3, 3)
    nc.gpsimd.tensor_tensor(T3v[:, 2], eav2[:], st["psw2a"][:], OP.subtract)

    # ---- stage 2: q_x(ph/2)* x t  (swap within pairs) ----
    TT(mcv[:, :], bc(csn[:, 0], 4), t4v[:, :], OP.mult)
    ms4d = st["ms"].rearrange("p (a b c) -> p a b c", a=2, c=Cw)
    t4sw = st["t4"].rearrange("p (a b c) -> p a b c", a=2, c=Cw)[:, :, ::-1]
    TT(ms4d, bc4d(ssn[:, 0]), t4sw, OP.mult)
    u4 = tt("u4", 4, tag="st")
    u4v = v(u4, 4)
    TT(u4v[:, 0:4:2], mcv[:, 0:4:2], msv[:, 0:4:2], OP.add)
    TT(u4v[:, 1:4:2], mcv[:, 1:4:2], msv[:, 1:4:2], OP.subtract)

    # ---- stage 3: q_y(th/2)* x u  (rotate-2) ----
    TT(mcv[:, :], bc(csn[:, 1], 4), u4v[:, :], OP.mult)
    ms4r = st["ms"].rearrange("p (a b c) -> p a b c", b=2, c=Cw)
    u4rot = st["u4"].rearrange("p (a b c) -> p a b c", b=2, c=Cw)[:, ::-1]
    TT(ms4r, bc4d(ssn[:, 1]), u4rot, OP.mult)
    a4 = tt("a4", 4, tag="st")
    a4v = v(a4, 4)
    TT(a4v[:, 0:4:3], mcv[:, 0:4:3], msv[:, 0:4:3], OP.add)
    TT(a4v[:, 1:3], mcv[:, 1:3], msv[:, 1:3], OP.subtract)

    # ---- products: P6 = (ab, ac, bd, cd, bb, cc) (UNdoubled) ----
    P6 = tt("P6", 6)
    P6v = v(P6, 6)
    TT(P6v[:, 0:2], bc(a4v[:, 0], 2), a4v[:, 1:3], OP.mult)
    TT(P6v[:, 2:4], a4v[:, 1:3], bc(a4v[:, 3], 2), OP.mult)
    TT(P6v[:, 4:6], a4v[:, 1:3], a4v[:, 1:3], OP.mult)

    # ---- M3 = (cd-ab, bd+ac, bb+cc) = (M12, M02, Sg) ----
    M3 = tt("M3", 3)
    M3v = v(M3, 3)
    TT(M3v[:, 0], P6v[:, 3], P6v[:, 0], OP.subtract)
    TT(M3v[:, 1:3], P6v[:, 2:5:2], P6v[:, 1:6:4], OP.add)

    # ---- s3 = (psw*M12, psw*M02, -psw2b*Sg) ----
    s3 = tt("s3", 3)
    s3v = v(s3, 3)
    TT(s3v[:, 0:2], bc(st["psw"][:], 2), M3v[:, 0:2], OP.mult)
    TT(s3v[:, 2], st["psw2b"][:], M3v[:, 2], OP.mult)

    # ---- T3[0:2] = (2wa*ac + wr*av1, 2wa*ab + wr*av0); GB rows; UV ----
    e13 = tt("e13", 2)
    nc.vector.tensor_scalar(e13[:], P6[:, 0 : 2 * Cw], 2.0 * fp["wa"], None,
                            OP.mult)
    TT(T3v[:, 1::-1], v(e13, 2)[:, :], v(st["eav01"], 2)[:, :], OP.add)
    GBv = v(st["GB"], 4)
    TT(GBv[:, 1:4], T3v[:, :], s3v[:, :], OP.subtract)
    UV = tt("UV", 4)
    UVv = v(UV, 4)
    TT(UVv[:, 0:2], GBv[:, 0:2], GBv[:, 2:4], OP.add)
    TT(UVv[:, 2:4], GBv[:, 0:2], GBv[:, 2:4], OP.subtract)


def _back(nc, mybir, io, st, ti, out2, fp, Cw):
    f32 = mybir.dt.float32
    OP = mybir.AluOpType
    c0 = st["c0"]

    out_t = io.tile([P, Cw * 4], f32, tag="out", name=f"out_{ti}")
    out3 = out_t.rearrange("p (c m) -> p c m", m=4)
    UVv = st["UV"].rearrange("p (k c) -> p k c", c=Cw)

    uidx = [0 if fp["sA"][r] > 0 else 2 for r in range(4)]
    vidx = [1 if fp["sB"][r] * fp["sC"][r] > 0 else 3 for r in range(4)]
    for r in range(4):
        op = OP.add if fp["sB"][r] > 0 else OP.subtract
        nc.gpsimd.tensor_tensor(out3[:, :, r], UVv[:, uidx[r]], UVv[:, vidx[r]],
                                op)
    # out-DMA triggered from gpsimd: keeps the SP engine free to prefetch
    # the next tiles' input DMAs instead of blocking behind this tile's outs
    nc.gpsimd.dma_start(out=out2[:, c0 * 4 : (c0 + Cw) * 4], in_=out_t[:])


def _build_nc(fp, reps=1, trace_sim=False, cols=None, tile_widths=None):
    import concourse.bass as bass
    import concourse.mybir as mybir
    from concourse.tile import TileContext

    f32 = mybir.dt.float32
    cdt = getattr(mybir.dt, COMPUTE_DT)
    ncols = COLS if cols is None else cols
    widths = TILE_WIDTHS if tile_widths is None else tile_widths

    nc = bass.Bass()

    # const AP for the pi/2 bias used by cos-via-sin
    cbias = nc.alloc_sbuf_tensor("const-f32-pio2", [128, 1], f32)
    nc.gpsimd.memset(cbias.ap(), _PIO2)
    nc.const_aps.aps[(f32, _PIO2)] = cbias.ap()
    nc.all_engine_barrier()

    shard = ncols * P
    rs = nc.declare_dram_parameter("root_state", [shard, 13], f32, isOutput=False)
    ct = nc.declare_dram_parameter("control_target", [shard, 4], f32, isOutput=False)
    out = nc.declare_dram_parameter("out", [shard, 4], f32, isOutput=True)
    rs2 = rs.rearrange("(p c) m -> p (c m)", p=P)
    ct2 = ct.rearrange("(p c) m -> p (c m)", p=P)
    out2 = out.rearrange("(p c) m -> p (c m)", p=P)

    assert PARTIAL_OK or sum(widths) == ncols

    # flat schedule of (tile_index, col0, width) across reps
    tiles = []
    for rep in range(reps):
        c0 = 0
        for Cw in widths:
            tiles.append((len(tiles), c0, Cw))
            c0 += Cw

    with TileContext(nc, trace_sim=trace_sim) as tc:
        with (
            tc.tile_pool(name="io", bufs=IO_BUFS) as io,
            tc.tile_pool(name="tmp", bufs=TMP_BUFS) as tmp,
        ):
            sts = {}
            n = len(tiles)
            for j in range(min(LOOKAHEAD, n)):
                ti, c0, Cw = tiles[j]
                sts[ti] = _front(nc, mybir, io, tmp, rs2, ct2, ti, c0, Cw, fp, cdt)
            for i in range(n):
                ti, c0, Cw = tiles[i]
                _body(nc, mybir, tmp, sts[ti], ti, Cw, fp, cdt)
                _back(nc, mybir, io, sts[ti], ti, out2, fp, Cw)
                del sts[ti]
                if i + LOOKAHEAD < n:
                    tj, cj, Cwj = tiles[i + LOOKAHEAD]
                    sts[tj] = _front(nc, mybir, io, tmp, rs2, ct2, tj, cj, Cwj,
                                     fp, cdt)
    return nc


# --------------------------------------------------------------------------
# Public entry point
# --------------------------------------------------------------------------
def kernel(root_state, control_target, mass, g, mixer, max_thrusts,
           gain_attitude, gain_angular_rate):
    root_state = np.ascontiguousarray(np.asarray(root_state, np.float32))
    control_target = np.ascontiguousarray(np.asarray(control_target, np.float32))
    assert root_state.shape == (B_TOTAL, 13), root_state.shape
    assert control_target.shape == (B_TOTAL, 4), control_target.shape

    fp = _fold_params(mass, g, mixer, max_thrusts, gain_attitude, gain_angular_rate)

    key = hashlib.sha256(
        repr(({k: v for k, v in fp.items() if k != "Wf"}, COMPUTE_DT,
              tuple(TILE_WIDTHS), IO_BUFS, TMP_BUFS, LOOKAHEAD, "v6")).encode()
    ).hexdigest()
    if key not in _CACHE:
        _install_bir_patch()
        _CACHE[key] = _build_nc(fp)
    nc = _CACHE[key]

    from concourse.bass_utils import run_bass_kernel_spmd

    rs_shards = root_state.reshape(N_CORES, SHARD, 13)
    ct_shards = control_target.reshape(N_CORES, SHARD, 4)
    in_maps = [
        {"root_state": rs_shards[i], "control_target": ct_shards[i]}
        for i in range(N_CORES)
    ]
    res = run_bass_kernel_spmd(nc, in_maps, core_ids=list(range(N_CORES)))
    return np.concatenate([res.results[i]["out"] for i in range(N_CORES)], axis=0)
